# revision 2
# baseline (speedup 1.0000x reference)
"""Trainium2 Bass kernel for nn_NetConvolve (sliding-window Conv1D + ReLU).

Math: out[b, s*497 + t, f] = relu( sum_{k,c} x[b, 256*s + t + k, c] * W[k,c,f] + b[f] )
for b in [0,32), s in [0,127), t in [0,497), k in [0,16), c in [0,2), f in [0,32).

Key observation: windows overlap (stride 256 < out_len 497), so 48.5% of the
output values are duplicates: out[b,s,t] == out[b,s+1,t-256] for t >= 256.
The device therefore computes the *full-signal* conv once per batch:
    y[b, n, f] = relu( sum_{k,c} x[b, n+k, c] * W[k,c,f] + b[f] ),  n in [0, 32753)
and the host reconstructs the windowed output with a gather (pure indexing,
part of the unshard step). This nearly halves both PE work and store traffic.

Device dataflow (per core; data parallel over batch, 4 batches/core):
  - A tile covers 128 partitions x T consecutive positions (T=48; tail tile
    T=16 so every DMA uses exactly 128 partitions - a 127-partition store
    was measured to serialize onto ONE of the 16 SDMA engines at ~26 GB/s,
    while 128-partition stores spread across all 16 at ~190-270 GB/s).
  - Per tile the conv is one matmul against a block-Toeplitz expansion of W
    (bias folded in via an all-ones column):
        y[m, (t,f)] = sum_{k'c} xw[m, k'c] * Wbig[k'c, (t,f)],  k' = t+k
    The tail tile (T=16) reuses columns [0, 512) of the same Wbig: rows
    k' >= 31 are zero there, so the stride-16 x window layout is consistent.
  - x windows land position-major via one PE transpose; outputs land
    position-major so each store is one [128, T*32] instruction into a
    fully contiguous HBM range.
  - Output is stored as fp16 (rounding adds <= 0.05% relative error on top
    of the fp32r compute); host converts to fp32.
"""

import numpy as np

B_FULL = 32
N_SAMP = 32768
C_IN = 2
KSIZE = 16
FILTERS = 32
WINDOW = 512
STRIDE = 256
S = 127                       # windows per batch
OUT_LEN = WINDOW - KSIZE + 1  # 497
NCORES = 8
BPC = B_FULL // NCORES        # batches per core = 4
SP = 128
T_FULL = 48                   # positions per partition, full tiles
T_TAIL = 16                   # positions per partition, tail tile
KDATA = (T_FULL + KSIZE - 1) * C_IN  # 126 data rows; row 126 = bias, 127 = 0
NPOS_PAD = 5 * SP * T_FULL + SP * T_TAIL  # 32768 stored positions per batch
NFLAT = N_SAMP * C_IN         # 65536
PAD = 8256                    # zero pad so every 128-partition load is in-bounds
NFLAT_PAD = NFLAT + PAD

# (n0, T) tiles covering positions [0, 32768) per batch
TILES = [(i * SP * T_FULL, T_FULL) for i in range(5)] + [(5 * SP * T_FULL, T_TAIL)]


def _build_wbig(W: np.ndarray, b: np.ndarray) -> np.ndarray:
    """Wbig[(k'*2 + c), (t*32 + f)] = W[k'-t, c, f] when 0 <= k'-t < 16 else 0.
    Row 126 holds the bias tiled per t; row 127 is zero (padding)."""
    Wbig = np.zeros((SP, T_FULL * FILTERS), np.float32)
    for t in range(T_FULL):
        for k in range(KSIZE):
            kp = t + k
            for c in range(C_IN):
                Wbig[kp * C_IN + c, t * FILTERS:(t + 1) * FILTERS] = W[k, c, :]
    Wbig[KDATA, :] = np.tile(np.asarray(b, np.float32), T_FULL)
    return Wbig


def _split_sync_waits(nc, limit=1):
    """This walrus build packs at most `limit` semaphore waits into one
    instruction's sync ctrl. Tile can emit more; move the excess onto
    same-engine NoOps inserted immediately before the instruction."""
    from concourse import mybir

    ctr = 0
    for fn in nc.m.functions:
        for bb in fn.blocks:
            new = []
            for inst in bb.instructions:
                si = inst.sync_info
                waits = list(si.on_wait) if (si and si.on_wait) else []
                if len(waits) > limit:
                    extra, keep = waits[:-limit], waits[-limit:]
                    for off in range(0, len(extra), limit):
                        nop = mybir.InstNoOp(
                            name=f"I-waitsplit-{ctr}",
                            engine=inst.engine,
                            ins=[],
                            outs=[],
                            sync_info=mybir.SyncInfo(
                                on_wait=extra[off:off + limit], on_update=[]
                            ),
                        )
                        ctr += 1
                        nc.register_instruction(nop, overwrite=True)
                        new.append(nop)
                    si.on_wait = keep
                new.append(inst)
            if ctr:
                bb.instructions[:] = new
    return nc


def _build_nc():
    import concourse.bass as bass
    from concourse import mybir, tile
    from contextlib import ExitStack

    f32 = mybir.dt.float32
    f32r = mybir.dt.float32r
    f16 = mybir.dt.float16

    nc = bass.Bass()
    x_h = nc.declare_dram_parameter("x", [BPC, NFLAT_PAD], f32r, isOutput=False)
    wbig_h = nc.declare_dram_parameter("wbig", [SP, T_FULL * FILTERS], f32r, isOutput=False)
    ident_h = nc.declare_dram_parameter("ident", [SP, SP], f32r, isOutput=False)
    ones_h = nc.declare_dram_parameter("ones", [SP, 1], f32r, isOutput=False)
    out_h = nc.declare_dram_parameter("out", [BPC, NPOS_PAD * FILTERS], f16, isOutput=True)

    with tile.TileContext(nc) as tc, ExitStack() as ctx:
        const_pool = ctx.enter_context(tc.tile_pool(name="const", bufs=1))
        xw_pool = ctx.enter_context(tc.tile_pool(name="xw", bufs=4))
        lhs_pool = ctx.enter_context(tc.tile_pool(name="lhs", bufs=3))
        outs_pool = ctx.enter_context(tc.tile_pool(name="outs", bufs=3))
        psT_pool = ctx.enter_context(tc.tile_pool(name="psT", bufs=2, space="PSUM"))
        psO_pool = ctx.enter_context(tc.tile_pool(name="psO", bufs=4, space="PSUM"))

        wbig_sb = const_pool.tile([SP, T_FULL * FILTERS], f32r)
        nc.scalar.dma_start(wbig_sb[:], wbig_h[:])
        ident_sb = const_pool.tile([SP, SP], f32r)
        nc.scalar.dma_start(ident_sb[:], ident_h[:])
        ones_sb = const_pool.tile([SP, 1], f32r)
        nc.scalar.dma_start(ones_sb[:], ones_h[:])

        relu_cnt = 0
        for b in range(BPC):
            for (n0, T) in TILES:
                # xw[m, j] = x_flat[b, (n0 + T*m)*2 + j]; j in [0,128)
                xw = xw_pool.tile([SP, 128], f32r)
                src = bass.AP(
                    tensor=x_h,
                    offset=b * NFLAT_PAD + n0 * C_IN,
                    ap=[[T * C_IN, SP], [1, 128]],
                )
                nc.scalar.dma_start(xw[:], src)
                # bias row: ones in column KDATA (=126)
                nc.vector.tensor_copy(xw[:, KDATA:KDATA + 1], ones_sb[:])

                # transpose -> lhsT [K=128 rows, 128 position-chunks]
                psT = psT_pool.tile([SP, SP], f32r)
                nc.tensor.transpose(psT[:], xw[:], ident_sb[:])
                lhsT = lhs_pool.tile([SP, SP], f32r)
                nc.vector.tensor_copy(lhsT[:], psT[:])

                ntot = T * FILTERS
                sbo = outs_pool.tile([SP, ntot], f16)
                n0c = 0
                while n0c < ntot:
                    n1c = min(ntot, n0c + 512)
                    pso = psO_pool.tile([SP, n1c - n0c], f32)
                    nc.tensor.matmul(
                        pso[:], lhsT[:], wbig_sb[:, n0c:n1c], start=True, stop=True
                    )
                    if relu_cnt % 3 == 1:
                        nc.scalar.activation(
                            sbo[:, n0c:n1c], pso[:],
                            mybir.ActivationFunctionType.Relu,
                        )
                    else:
                        nc.vector.tensor_scalar_max(sbo[:, n0c:n1c], pso[:], 0.0)
                    relu_cnt += 1
                    n0c = n1c

                # contiguous [128, T*32] fp16 store
                dst = bass.AP(
                    tensor=out_h,
                    offset=b * NPOS_PAD * FILTERS + n0 * FILTERS,
                    ap=[[ntot, SP], [1, ntot]],
                )
                nc.sync.dma_start(dst, sbo[:])

    _split_sync_waits(nc)
    nc.finalize()
    return nc


def _prep_inputs(x: np.ndarray, W: np.ndarray, b: np.ndarray):
    x = np.ascontiguousarray(np.asarray(x, np.float32))
    Wbig = _build_wbig(np.asarray(W, np.float32), np.asarray(b, np.float32))
    ident = np.eye(SP, dtype=np.float32)
    xf = x.reshape(B_FULL, NFLAT)
    xpad = np.zeros((B_FULL, NFLAT_PAD), np.float32)
    xpad[:, :NFLAT] = xf
    ones = np.ones((SP, 1), np.float32)
    in_maps = [
        {
            "x": np.ascontiguousarray(xpad[c * BPC:(c + 1) * BPC]),
            "wbig": Wbig,
            "ident": ident,
            "ones": ones,
        }
        for c in range(NCORES)
    ]
    return in_maps


def _gather_windows(yfull: np.ndarray) -> np.ndarray:
    """yfull [B, 32768, F] (device layout) -> out [B, S*OUT_LEN, F] fp32."""
    idx = (np.arange(S)[:, None] * STRIDE + np.arange(OUT_LEN)[None, :]).ravel()
    return yfull[:, idx, :].astype(np.float32)


def kernel(x: np.ndarray, W: np.ndarray, b: np.ndarray) -> np.ndarray:
    from concourse.bass_utils import run_bass_kernel_spmd

    nc = _build_nc()
    in_maps = _prep_inputs(x, W, b)
    res = run_bass_kernel_spmd(nc, in_maps, list(range(NCORES))).results
    yfull = np.concatenate(
        [np.asarray(res[c]["out"]) for c in range(NCORES)], axis=0
    ).reshape(B_FULL, NPOS_PAD, FILTERS)
    return _gather_windows(yfull)
